# revision 6
# baseline (speedup 1.0000x reference)
import sys

sys.path.insert(0, "/opt/trn_rl_repo")

import numpy as np
import ml_dtypes

from concourse import bass, tile, bacc
from concourse.bass_utils import run_bass_kernel_spmd

WORLD, M, N, K_LOCAL = 8, 8192, 2048, 256
M_PER = M // WORLD  # 1024
KT = WORLD * K_LOCAL  # 2048 total contraction dim
NK = KT // 128  # 16 k-tiles
NCH = 512
NNC = N // NCH  # 4 n-chunks
NM = M_PER // 128  # 8 m-tiles
F32 = bass.mybir.dt.float32
BF16 = bass.mybir.dt.bfloat16

# k-tile groups per batched input DMA: fine-grained up front so the first
# matmul chains can start, coarse later to keep DMA-issue count low.
K_GROUPS = [(0, 1), (1, 2), (2, 4), (4, 8), (8, 12), (12, 16)]

_NC = None


def _build():
    # Sharding: the leading "rank" axis of A/weight is just a K-shard index,
    # so instead of K-parallel + reduce-scatter, shard over M: core r does the
    # full K=2048 reduction for its own [1024, 2048] output block. No
    # collective, no 64MiB partial staging.
    nc = bacc.Bacc(None, target_bir_lowering=False, num_devices=WORLD)
    At = nc.dram_tensor("a_t", [KT, M_PER], BF16, kind="ExternalInput")
    Wt = nc.dram_tensor("w_t", [NNC * KT, NCH], BF16, kind="ExternalInput")
    out = nc.dram_tensor("out", [M_PER, N], F32, kind="ExternalOutput")

    with tile.TileContext(nc) as tc:
        with (
            tc.tile_pool(name="ab", bufs=1) as ab,
            tc.tile_pool(name="wb", bufs=1) as wb,
            tc.tile_pool(name="ob", bufs=4) as ob,
            tc.tile_pool(name="sc", bufs=1) as sc,
            tc.tile_pool(name="ps", bufs=8, space="PSUM") as ps,
        ):
            A_sb = ab.tile([128, NK, M_PER], BF16)  # 32 KB/partition
            W_sb = wb.tile([128, NNC, NK, NCH], BF16)  # 64 KB/partition

            # PE pre-warm: HAM needs ~3.4us of sustained PE activity before it
            # ungates the 2.4 GHz clock. Run dummy matmuls on a zeroed scratch
            # tile (no DMA dependency) so the warm-up happens while the first
            # input tiles are still in flight.
            scratch = sc.tile([128, NCH + 128], BF16)
            nc.gpsimd.memset(scratch[:], 0.0)
            for _ in range(12):
                warm = ps.tile([128, NCH], F32, tag="acc")
                nc.tensor.matmul(
                    warm[:],
                    scratch[:, NCH : NCH + 128],
                    scratch[:, 0:NCH],
                    start=True,
                    stop=True,
                )

            # Batched loads, split across the two HWDGE issue queues so A and
            # W issue in parallel: A k-groups on sync, W on scalar.
            for k0, k1 in K_GROUPS:
                nc.sync.dma_start(
                    A_sb[:, k0:k1, :],
                    At[k0 * 128 : k1 * 128, :].rearrange(
                        "(t p) m -> p t m", p=128
                    ),
                )
                nc.scalar.dma_start(
                    W_sb[:, 0, k0:k1, :],
                    Wt[k0 * 128 : k1 * 128, :].rearrange(
                        "(t p) n -> p t n", p=128
                    ),
                )
            for nci in range(1, NNC):
                nc.scalar.dma_start(
                    W_sb[:, nci, :, :],
                    Wt[nci * KT : (nci + 1) * KT, :].rearrange(
                        "(t p) n -> p t n", p=128
                    ),
                )

            for nci in range(NNC):
                for mi in range(NM):
                    acc = ps.tile([128, NCH], F32)
                    for ki in range(NK):
                        nc.tensor.matmul(
                            acc[:],
                            A_sb[:, ki, mi * 128 : (mi + 1) * 128],
                            W_sb[:, nci, ki, :],
                            start=(ki == 0),
                            stop=(ki == NK - 1),
                        )
                    row = ob.tile([128, NCH], F32)
                    nc.vector.tensor_copy(row[:], acc[:])
                    # sync queue is free once the A loads have issued, so
                    # outputs can't head-of-line-block the W loads (scalar)
                    nc.sync.dma_start(
                        out[mi * 128 : (mi + 1) * 128, nci * NCH : (nci + 1) * NCH],
                        row[:],
                    )
    nc.compile()
    return nc


def _prep(A, weight):
    A = np.asarray(A, dtype=np.float32)
    weight = np.asarray(weight, dtype=np.float32)
    # weight [world, N, K_local] -> K-major [KT, N], then n-chunk-major
    # [NNC, KT, NCH] flattened so each k-tile slab is contiguous.
    wt = weight.transpose(0, 2, 1).reshape(KT, N)
    wt4 = (
        np.ascontiguousarray(wt.reshape(KT, NNC, NCH).transpose(1, 0, 2))
        .reshape(NNC * KT, NCH)
        .astype(ml_dtypes.bfloat16)
    )
    in_maps = []
    for r in range(WORLD):
        strip = A[:, r * M_PER : (r + 1) * M_PER, :]  # [world, 1024, K_local]
        at = (
            np.ascontiguousarray(strip.transpose(0, 2, 1))
            .reshape(KT, M_PER)
            .astype(ml_dtypes.bfloat16)
        )
        in_maps.append({"a_t": at, "w_t": wt4})
    return in_maps


def kernel(A, weight, _trace=False):
    global _NC
    if _NC is None:
        _NC = _build()
    in_maps = _prep(A, weight)
    res = run_bass_kernel_spmd(
        _NC, in_maps, core_ids=list(range(WORLD)), trace=_trace
    )
    out = np.stack([res.results[r]["out"] for r in range(WORLD)], axis=0)
    if _trace:
        return out, res
    return out


# revision 10
# speedup vs baseline: 1.0237x; 1.0237x over previous
import sys

sys.path.insert(0, "/opt/trn_rl_repo")

import numpy as np
import ml_dtypes

from concourse import bass, tile, bacc
from concourse.bass_utils import run_bass_kernel_spmd

WORLD, M, N, K_LOCAL = 8, 8192, 2048, 256
M_PER = M // WORLD  # 1024
KT = WORLD * K_LOCAL  # 2048 total contraction dim
NK = KT // 128  # 16 k-tiles
NCH = 512
NNC = N // NCH  # 4 n-chunks
NM = M_PER // 128  # 8 m-tiles
F32 = bass.mybir.dt.float32
BF16 = bass.mybir.dt.bfloat16

# k-tile groups per batched input DMA: fine-grained up front so the first
# matmul chains can start, coarse later to keep DMA-issue count low.
K_GROUPS = [(0, 1), (1, 2), (2, 3), (3, 4), (4, 6), (6, 8), (8, 10), (10, 12), (12, 16)]

_NC = None


def _build():
    # Sharding: the leading "rank" axis of A/weight is just a K-shard index,
    # so instead of K-parallel + reduce-scatter, shard over M: core r does the
    # full K=2048 reduction for its own [1024, 2048] output block. No
    # collective, no 64MiB partial staging.
    nc = bacc.Bacc(None, target_bir_lowering=False, num_devices=WORLD)
    At = nc.dram_tensor("a_t", [KT, M_PER], BF16, kind="ExternalInput")
    Wt = nc.dram_tensor("w_t", [NNC * KT, NCH], BF16, kind="ExternalInput")
    out = nc.dram_tensor("out", [M_PER, N], F32, kind="ExternalOutput")

    with tile.TileContext(nc) as tc:
        with (
            tc.tile_pool(name="ab", bufs=1) as ab,
            tc.tile_pool(name="wb", bufs=1) as wb,
            tc.tile_pool(name="ob", bufs=4) as ob,
            tc.tile_pool(name="ps", bufs=8, space="PSUM") as ps,
        ):
            A_sb = ab.tile([128, NK, M_PER], BF16)  # 32 KB/partition
            W_sb = wb.tile([128, NNC, NK, NCH], BF16)  # 64 KB/partition

            # Batched loads: A k-groups and W chunk-0 k-groups interleaved
            # (what the first chains consume, in consumption order), then the
            # remaining W chunks whole.
            for k0, k1 in K_GROUPS:
                nc.sync.dma_start(
                    A_sb[:, k0:k1, :],
                    At[k0 * 128 : k1 * 128, :].rearrange(
                        "(t p) m -> p t m", p=128
                    ),
                )
                nc.sync.dma_start(
                    W_sb[:, 0, k0:k1, :],
                    Wt[k0 * 128 : k1 * 128, :].rearrange(
                        "(t p) n -> p t n", p=128
                    ),
                )
            for nci in range(1, NNC):
                nc.sync.dma_start(
                    W_sb[:, nci, :, :],
                    Wt[nci * KT : (nci + 1) * KT, :].rearrange(
                        "(t p) n -> p t n", p=128
                    ),
                )

            for nci in range(NNC):
                for mi in range(NM):
                    acc = ps.tile([128, NCH], F32)
                    for ki in range(NK):
                        nc.tensor.matmul(
                            acc[:],
                            A_sb[:, ki, mi * 128 : (mi + 1) * 128],
                            W_sb[:, nci, ki, :],
                            start=(ki == 0),
                            stop=(ki == NK - 1),
                        )
                    row = ob.tile([128, NCH], F32)
                    nc.vector.tensor_copy(row[:], acc[:])
                    # outputs go out on the scalar engine's DMA queue so they
                    # never head-of-line-block input loads on the sync queue
                    nc.scalar.dma_start(
                        out[mi * 128 : (mi + 1) * 128, nci * NCH : (nci + 1) * NCH],
                        row[:],
                    )
    nc.compile()
    return nc


def _prep(A, weight):
    A = np.asarray(A, dtype=np.float32)
    weight = np.asarray(weight, dtype=np.float32)
    # weight [world, N, K_local] -> K-major [KT, N], then n-chunk-major
    # [NNC, KT, NCH] flattened so each k-tile slab is contiguous.
    wt = weight.transpose(0, 2, 1).reshape(KT, N)
    wt4 = (
        np.ascontiguousarray(wt.reshape(KT, NNC, NCH).transpose(1, 0, 2))
        .reshape(NNC * KT, NCH)
        .astype(ml_dtypes.bfloat16)
    )
    in_maps = []
    for r in range(WORLD):
        strip = A[:, r * M_PER : (r + 1) * M_PER, :]  # [world, 1024, K_local]
        at = (
            np.ascontiguousarray(strip.transpose(0, 2, 1))
            .reshape(KT, M_PER)
            .astype(ml_dtypes.bfloat16)
        )
        in_maps.append({"a_t": at, "w_t": wt4})
    return in_maps


def kernel(A, weight, _trace=False):
    global _NC
    if _NC is None:
        _NC = _build()
    in_maps = _prep(A, weight)
    res = run_bass_kernel_spmd(
        _NC, in_maps, core_ids=list(range(WORLD)), trace=_trace
    )
    out = np.stack([res.results[r]["out"] for r in range(WORLD)], axis=0)
    if _trace:
        return out, res
    return out


# revision 11
# speedup vs baseline: 1.0393x; 1.0152x over previous
import sys

sys.path.insert(0, "/opt/trn_rl_repo")

import numpy as np
import ml_dtypes

from concourse import bass, tile, bacc
from concourse.bass_utils import run_bass_kernel_spmd

WORLD, M, N, K_LOCAL = 8, 8192, 2048, 256
M_PER = M // WORLD  # 1024
KT = WORLD * K_LOCAL  # 2048 total contraction dim
NK = KT // 128  # 16 k-tiles
NCH = 512
NNC = N // NCH  # 4 n-chunks
NM = M_PER // 128  # 8 m-tiles
F32 = bass.mybir.dt.float32
BF16 = bass.mybir.dt.bfloat16

# k-tile groups per batched input DMA: fine-grained up front so the first
# matmul chains can start, coarse later to keep DMA-issue count low.
K_GROUPS = [(0, 1), (1, 2), (2, 4), (4, 8), (8, 12), (12, 16)]

_NC = None


def _build():
    # Sharding: the leading "rank" axis of A/weight is just a K-shard index,
    # so instead of K-parallel + reduce-scatter, shard over M: core r does the
    # full K=2048 reduction for its own [1024, 2048] output block. No
    # collective, no 64MiB partial staging.
    nc = bacc.Bacc(None, target_bir_lowering=False, num_devices=WORLD)
    At = nc.dram_tensor("a_t", [KT, M_PER], BF16, kind="ExternalInput")
    Wt = nc.dram_tensor("w_t", [NNC * KT, NCH], BF16, kind="ExternalInput")
    out = nc.dram_tensor("out", [M_PER, N], F32, kind="ExternalOutput")

    with tile.TileContext(nc) as tc:
        with (
            tc.tile_pool(name="ab", bufs=1) as ab,
            tc.tile_pool(name="wb", bufs=1) as wb,
            tc.tile_pool(name="ob", bufs=4) as ob,
            tc.tile_pool(name="ps", bufs=8, space="PSUM") as ps,
        ):
            A_sb = ab.tile([128, NK, M_PER], BF16)  # 32 KB/partition
            W_sb = wb.tile([128, NNC, NK, NCH], BF16)  # 64 KB/partition

            # Batched loads: A k-groups and W chunk-0 k-groups interleaved
            # (what the first chains consume, in consumption order), then the
            # remaining W chunks whole.
            for k0, k1 in K_GROUPS:
                nc.sync.dma_start(
                    A_sb[:, k0:k1, :],
                    At[k0 * 128 : k1 * 128, :].rearrange(
                        "(t p) m -> p t m", p=128
                    ),
                )
                nc.sync.dma_start(
                    W_sb[:, 0, k0:k1, :],
                    Wt[k0 * 128 : k1 * 128, :].rearrange(
                        "(t p) n -> p t n", p=128
                    ),
                )
            for nci in range(1, NNC):
                nc.sync.dma_start(
                    W_sb[:, nci, :, :],
                    Wt[nci * KT : (nci + 1) * KT, :].rearrange(
                        "(t p) n -> p t n", p=128
                    ),
                )

            for nci in range(NNC):
                for mi in range(NM):
                    acc = ps.tile([128, NCH], F32)
                    for ki in range(NK):
                        nc.tensor.matmul(
                            acc[:],
                            A_sb[:, ki, mi * 128 : (mi + 1) * 128],
                            W_sb[:, nci, ki, :],
                            start=(ki == 0),
                            stop=(ki == NK - 1),
                        )
                    row = ob.tile([128, NCH], F32)
                    nc.vector.tensor_copy(row[:], acc[:])
                    # outputs go out on the scalar engine's DMA queue so they
                    # never head-of-line-block input loads on the sync queue
                    nc.scalar.dma_start(
                        out[mi * 128 : (mi + 1) * 128, nci * NCH : (nci + 1) * NCH],
                        row[:],
                    )
    nc.compile()
    return nc


def _prep(A, weight):
    A = np.asarray(A, dtype=np.float32)
    weight = np.asarray(weight, dtype=np.float32)
    # weight [world, N, K_local] -> K-major [KT, N], then n-chunk-major
    # [NNC, KT, NCH] flattened so each k-tile slab is contiguous.
    wt = weight.transpose(0, 2, 1).reshape(KT, N)
    wt4 = (
        np.ascontiguousarray(wt.reshape(KT, NNC, NCH).transpose(1, 0, 2))
        .reshape(NNC * KT, NCH)
        .astype(ml_dtypes.bfloat16)
    )
    in_maps = []
    for r in range(WORLD):
        strip = A[:, r * M_PER : (r + 1) * M_PER, :]  # [world, 1024, K_local]
        at = (
            np.ascontiguousarray(strip.transpose(0, 2, 1))
            .reshape(KT, M_PER)
            .astype(ml_dtypes.bfloat16)
        )
        in_maps.append({"a_t": at, "w_t": wt4})
    return in_maps


def kernel(A, weight, _trace=False):
    global _NC
    if _NC is None:
        _NC = _build()
    in_maps = _prep(A, weight)
    res = run_bass_kernel_spmd(
        _NC, in_maps, core_ids=list(range(WORLD)), trace=_trace
    )
    out = np.stack([res.results[r]["out"] for r in range(WORLD)], axis=0)
    if _trace:
        return out, res
    return out


# revision 12
# speedup vs baseline: 1.0446x; 1.0051x over previous
import sys

sys.path.insert(0, "/opt/trn_rl_repo")

import numpy as np
import ml_dtypes

from concourse import bass, tile, bacc
from concourse.bass_utils import run_bass_kernel_spmd

WORLD, M, N, K_LOCAL = 8, 8192, 2048, 256
M_PER = M // WORLD  # 1024
KT = WORLD * K_LOCAL  # 2048 total contraction dim
NK = KT // 128  # 16 k-tiles
NCH = 512
NNC = N // NCH  # 4 n-chunks
NM = M_PER // 128  # 8 m-tiles
F32 = bass.mybir.dt.float32
BF16 = bass.mybir.dt.bfloat16

# k-tile groups per batched input DMA: fine-grained up front so the first
# matmul chains can start, coarse later to keep DMA-issue count low.
K_GROUPS = [(0, 1), (1, 2), (2, 4), (4, 8), (8, 12), (12, 16)]

_NC = None


def _build():
    # Sharding: the leading "rank" axis of A/weight is just a K-shard index,
    # so instead of K-parallel + reduce-scatter, shard over M: core r does the
    # full K=2048 reduction for its own [1024, 2048] output block. No
    # collective, no 64MiB partial staging.
    nc = bacc.Bacc(None, target_bir_lowering=False, num_devices=WORLD)
    At = nc.dram_tensor("a_t", [KT, M_PER], BF16, kind="ExternalInput")
    Wt = nc.dram_tensor("w_t", [NNC * KT, NCH], BF16, kind="ExternalInput")
    out = nc.dram_tensor("out", [M_PER, N], F32, kind="ExternalOutput")

    with tile.TileContext(nc) as tc:
        with (
            tc.tile_pool(name="ab", bufs=1) as ab,
            tc.tile_pool(name="wb", bufs=1) as wb,
            tc.tile_pool(name="ob", bufs=4) as ob,
            tc.tile_pool(name="ps", bufs=8, space="PSUM") as ps,
        ):
            A_sb = ab.tile([128, NK, M_PER], BF16)  # 32 KB/partition
            W_sb = wb.tile([128, NNC, NK, NCH], BF16)  # 64 KB/partition

            # Batched loads: A k-groups and W chunk-0 k-groups interleaved
            # (what the first chains consume, in consumption order), then the
            # remaining W chunks whole.
            for k0, k1 in K_GROUPS:
                nc.sync.dma_start(
                    A_sb[:, k0:k1, :],
                    At[k0 * 128 : k1 * 128, :].rearrange(
                        "(t p) m -> p t m", p=128
                    ),
                )
                nc.sync.dma_start(
                    W_sb[:, 0, k0:k1, :],
                    Wt[k0 * 128 : k1 * 128, :].rearrange(
                        "(t p) n -> p t n", p=128
                    ),
                )
            # W chunks 1-3 aren't consumed until ~40/70/100us in, but the DMA
            # rings would otherwise start them immediately and steal HBM
            # bandwidth from the A/W0 k-tiles the first chains are waiting on.
            # A 1-element sliver copy into each chunk's first tile creates a
            # WAR dep that holds chunk nci's transfer until chunk nci-1 (and
            # the tail of A) has landed.
            for nci in range(1, NNC):
                prev = (
                    A_sb[:, NK - 2, 0:1]
                    if nci == 1
                    else W_sb[:, nci - 1, NK - 1, 0:1]
                )
                nc.vector.tensor_copy(W_sb[:, nci, 0, 0:1], prev)
                nc.sync.dma_start(
                    W_sb[:, nci, :, :],
                    Wt[nci * KT : (nci + 1) * KT, :].rearrange(
                        "(t p) n -> p t n", p=128
                    ),
                )

            for nci in range(NNC):
                for mi in range(NM):
                    acc = ps.tile([128, NCH], F32)
                    for ki in range(NK):
                        nc.tensor.matmul(
                            acc[:],
                            A_sb[:, ki, mi * 128 : (mi + 1) * 128],
                            W_sb[:, nci, ki, :],
                            start=(ki == 0),
                            stop=(ki == NK - 1),
                        )
                    row = ob.tile([128, NCH], F32)
                    nc.vector.tensor_copy(row[:], acc[:])
                    # outputs go out on the scalar engine's DMA queue so they
                    # never head-of-line-block input loads on the sync queue
                    nc.scalar.dma_start(
                        out[mi * 128 : (mi + 1) * 128, nci * NCH : (nci + 1) * NCH],
                        row[:],
                    )
    nc.compile()
    return nc


def _prep(A, weight):
    A = np.asarray(A, dtype=np.float32)
    weight = np.asarray(weight, dtype=np.float32)
    # weight [world, N, K_local] -> K-major [KT, N], then n-chunk-major
    # [NNC, KT, NCH] flattened so each k-tile slab is contiguous.
    wt = weight.transpose(0, 2, 1).reshape(KT, N)
    wt4 = (
        np.ascontiguousarray(wt.reshape(KT, NNC, NCH).transpose(1, 0, 2))
        .reshape(NNC * KT, NCH)
        .astype(ml_dtypes.bfloat16)
    )
    in_maps = []
    for r in range(WORLD):
        strip = A[:, r * M_PER : (r + 1) * M_PER, :]  # [world, 1024, K_local]
        at = (
            np.ascontiguousarray(strip.transpose(0, 2, 1))
            .reshape(KT, M_PER)
            .astype(ml_dtypes.bfloat16)
        )
        in_maps.append({"a_t": at, "w_t": wt4})
    return in_maps


def kernel(A, weight, _trace=False):
    global _NC
    if _NC is None:
        _NC = _build()
    in_maps = _prep(A, weight)
    res = run_bass_kernel_spmd(
        _NC, in_maps, core_ids=list(range(WORLD)), trace=_trace
    )
    out = np.stack([res.results[r]["out"] for r in range(WORLD)], axis=0)
    if _trace:
        return out, res
    return out
